# revision 8
# baseline (speedup 1.0000x reference)
"""Trainium2 Bass kernel for one Mixture-of-Memory (MOM) step.

Reference computation (per memory slot n, batch b):
    outer      = key[b] (outer) value[b]                    # [Dk, Dv]
    new_states[n,b] = lam[b,n] * states[n,b] + (rho[b,n]*alpha[b]) * outer
    readouts[n,b]   = query[b]^T @ new_states[n,b]          # [Dv]

Sharding: batch dim B=64 split across 8 NeuronCores (8 batches/core).
Every op is batch-pointwise, so there is no cross-core communication.

Per-core layout (pair = n*8 + b_local, n-major so it matches the DRAM
order of states/readouts):
  - states arrive as [64, 256, 256]; each (n,b) matrix is loaded as an
    SBUF tile slice [128 partitions, 2*256] with partition = k%128 and
    free = (k_half, v).  One 2 MB DMA per n covers all 8 local batches.
  - The k (outer) x v products for the 8 local batches are computed once
    on the PE (contraction dim 1) and kept in SBUF.
  - Update: ACT does lam*M_old (activation Copy with per-partition
    scale), DVE adds w*outer on top (scalar_tensor_tensor).
  - Readout: 2 accumulating matmuls per pair with lhsT = q^T column
    (PSUM [1,256]), copied to an SBUF staging row on ACT.
"""

import numpy as np

N_MEM = 8
B = 64
DK = 256
DV = 256
N_CORES = 8
BL = B // N_CORES  # local batch per core = 8
PAIRS = N_MEM * BL  # 64
P = 128  # SBUF partitions
KH = DK // P  # 2 k-halves

_CACHE = {}


def _build_nc():
    import concourse.masks as masks
    import concourse.mybir as mybir
    from concourse import bacc
    from concourse.tile import TileContext

    f32 = mybir.dt.float32
    nc = bacc.Bacc("TRN2", target_bir_lowering=False, debug=False,
                   num_devices=N_CORES)

    states_t = nc.dram_tensor("states", [PAIRS, DK, DV], f32, kind="ExternalInput")
    key_t = nc.dram_tensor("key", [BL, DK], f32, kind="ExternalInput")
    value_t = nc.dram_tensor("value", [BL, DV], f32, kind="ExternalInput")
    alpha_t = nc.dram_tensor("alpha", [BL, 1], f32, kind="ExternalInput")
    rho_t = nc.dram_tensor("rho", [BL, N_MEM], f32, kind="ExternalInput")
    lam_t = nc.dram_tensor("lam", [BL, N_MEM], f32, kind="ExternalInput")
    query_t = nc.dram_tensor("query", [BL, DK], f32, kind="ExternalInput")

    ns_t = nc.dram_tensor("new_states", [PAIRS, DK, DV], f32, kind="ExternalOutput")
    ro_t = nc.dram_tensor("readouts", [PAIRS, DV], f32, kind="ExternalOutput")

    mult = mybir.AluOpType.mult
    add = mybir.AluOpType.add
    act_copy = mybir.ActivationFunctionType.Copy

    with TileContext(nc) as tc:
        with (
            tc.tile_pool(name="const", bufs=1) as cpool,
            tc.tile_pool(name="tin", bufs=3) as in_pool,
            tc.tile_pool(name="tout", bufs=3) as out_pool,
            tc.tile_pool(name="rrow", bufs=2) as r_pool,
            tc.tile_pool(name="ps_bc", bufs=1, space="PSUM") as ps_bc_pool,
            tc.tile_pool(name="ps_outer", bufs=2, space="PSUM") as ps_outer_pool,
            tc.tile_pool(name="ps_r", bufs=4, space="PSUM") as ps_r_pool,
        ):
            # ---- small-input prep -------------------------------------
            key_row = cpool.tile([1, BL * DK], f32)
            nc.sync.dma_start(out=key_row[0:1, :],
                              in_=key_t[:].rearrange("b k -> (b k)")[None, :])
            value_row = cpool.tile([1, BL * DV], f32)
            nc.sync.dma_start(out=value_row[0:1, :],
                              in_=value_t[:].rearrange("b k -> (b k)")[None, :])
            rho_row = cpool.tile([1, PAIRS], f32)
            nc.sync.dma_start(out=rho_row[0:1, :],
                              in_=rho_t[:].rearrange("b n -> (b n)")[None, :])
            lam_row = cpool.tile([1, PAIRS], f32)
            nc.sync.dma_start(out=lam_row[0:1, :],
                              in_=lam_t[:].rearrange("b n -> (b n)")[None, :])
            alpha_row = cpool.tile([1, BL], f32)
            nc.sync.dma_start(out=alpha_row[0:1, :],
                              in_=alpha_t[:].rearrange("b o -> (o b)")[None, :])
            # q^T: [128, kh*8+b] = query[b, kh*128+p], via PE transpose
            q_sb = cpool.tile([BL, DK], f32)
            nc.sync.dma_start(out=q_sb[:, :], in_=query_t[:, :])
            id8 = cpool.tile([BL, BL], f32)
            masks.make_identity(nc, id8[:, :])
            qT = cpool.tile([P, KH * BL], f32)
            ps_q = ps_bc_pool.tile([P, KH * BL], f32, tag="ps_q")
            for kh in range(KH):
                nc.tensor.transpose(ps_q[:, kh * BL:(kh + 1) * BL],
                                    q_sb[0:BL, kh * P:(kh + 1) * P],
                                    id8[0:BL, 0:BL])
            nc.vector.tensor_copy(out=qT[:, :], in_=ps_q[:, :])

            ones_row = cpool.tile([1, P], f32)
            nc.vector.memset(ones_row[0:1, :], 1.0)

            # w[b,n] = rho[b,n] * alpha[b]  (all on partition 0)
            w_row = cpool.tile([1, PAIRS], f32)
            for b in range(BL):
                nc.vector.tensor_scalar_mul(
                    w_row[0:1, b * N_MEM:(b + 1) * N_MEM],
                    rho_row[0:1, b * N_MEM:(b + 1) * N_MEM],
                    alpha_row[0:1, b:b + 1],
                )

            # broadcast w and lam to all 128 partitions via ones-matmul
            ps_bc = ps_bc_pool.tile([P, 2 * PAIRS], f32)
            nc.tensor.matmul(ps_bc[:, 0:PAIRS], lhsT=ones_row[0:1, :],
                             rhs=w_row[0:1, :], start=True, stop=True)
            nc.tensor.matmul(ps_bc[:, PAIRS:2 * PAIRS], lhsT=ones_row[0:1, :],
                             rhs=lam_row[0:1, :], start=True, stop=True)
            bc_sb = cpool.tile([P, 2 * PAIRS], f32)
            nc.vector.tensor_copy(out=bc_sb[:, :], in_=ps_bc[:, :])

            # unscaled outer products per local batch: [128, (b, kh*v)]
            outer_sb = cpool.tile([P, BL * KH * DV], f32)
            for b in range(BL):
                ps_o = ps_outer_pool.tile([P, KH * DV], f32)
                for kh in range(KH):
                    nc.tensor.matmul(
                        ps_o[:, kh * DV:(kh + 1) * DV],
                        lhsT=key_row[0:1, b * DK + kh * P: b * DK + (kh + 1) * P],
                        rhs=value_row[0:1, b * DV:(b + 1) * DV],
                        start=True, stop=True)
                nc.vector.tensor_copy(out=outer_sb[:, b * KH * DV:(b + 1) * KH * DV],
                                      in_=ps_o[:, :])

            # ---- main loop over memory slots --------------------------
            for n in range(N_MEM):
                tin = in_pool.tile([P, BL * KH * DV], f32)
                nc.sync.dma_start(
                    out=tin[:, :].rearrange("p (x v) -> p x v", v=DV),
                    in_=states_t[n * BL:(n + 1) * BL]
                        .rearrange("b (kh p) v -> p (b kh) v", p=P))
                tout = out_pool.tile([P, BL * KH * DV], f32)
                rrow = r_pool.tile([1, BL * DV], f32)
                for b in range(BL):
                    col = b * N_MEM + n
                    sl = slice(b * KH * DV, (b + 1) * KH * DV)
                    # tout = lam * M_old  (ScalarE)
                    nc.scalar.activation(tout[:, sl], tin[:, sl], act_copy,
                                         scale=bc_sb[:, PAIRS + col:PAIRS + col + 1])
                    # tout += w * outer  (VectorE)
                    nc.vector.scalar_tensor_tensor(
                        out=tout[:, sl], in0=outer_sb[:, sl],
                        scalar=bc_sb[:, col:col + 1], in1=tout[:, sl],
                        op0=mult, op1=add)
                    # readout: q^T @ M_new  -> psum [1, 256]
                    ps_r = ps_r_pool.tile([1, DV], f32)
                    for kh in range(KH):
                        nc.tensor.matmul(
                            ps_r[0:1, :],
                            lhsT=qT[:, kh * BL + b: kh * BL + b + 1],
                            rhs=tout[:, b * KH * DV + kh * DV: b * KH * DV + (kh + 1) * DV],
                            start=(kh == 0), stop=(kh == KH - 1))
                    nc.scalar.copy(out=rrow[0:1, b * DV:(b + 1) * DV],
                                   in_=ps_r[0:1, :])
                nc.sync.dma_start(
                    out=ns_t[n * BL:(n + 1) * BL]
                        .rearrange("b (kh p) v -> p (b kh) v", p=P),
                    in_=tout[:, :].rearrange("p (x v) -> p x v", v=DV))
                nc.sync.dma_start(
                    out=ro_t[n * BL:(n + 1) * BL].rearrange("b v -> (b v)")[None, :],
                    in_=rrow[0:1, :])
    nc.compile()
    return nc


def _get_nc():
    if "nc" not in _CACHE:
        _CACHE["nc"] = _build_nc()
    return _CACHE["nc"]


def _in_maps(inputs):
    states = np.ascontiguousarray(np.asarray(inputs["states"], dtype=np.float32))
    key = np.asarray(inputs["key"], dtype=np.float32)
    value = np.asarray(inputs["value"], dtype=np.float32)
    alpha = np.asarray(inputs["alpha"], dtype=np.float32)
    rho = np.asarray(inputs["rho"], dtype=np.float32)
    lam = np.asarray(inputs["lam"], dtype=np.float32)
    query = np.asarray(inputs["query"], dtype=np.float32)

    maps = []
    for c in range(N_CORES):
        sl = slice(c * BL, (c + 1) * BL)
        maps.append({
            "states": np.ascontiguousarray(states[:, sl]).reshape(PAIRS, DK, DV),
            "key": np.ascontiguousarray(key[sl]),
            "value": np.ascontiguousarray(value[sl]),
            "alpha": np.ascontiguousarray(alpha[sl]),
            "rho": np.ascontiguousarray(rho[sl]),
            "lam": np.ascontiguousarray(lam[sl]),
            "query": np.ascontiguousarray(query[sl]),
        })
    return maps


def _run(in_maps, **kwargs):
    from concourse.bass_utils import run_bass_kernel_spmd
    nc = _get_nc()
    return run_bass_kernel_spmd(nc, in_maps, core_ids=list(range(N_CORES)),
                                **kwargs)


def kernel(**inputs):
    res = _run(_in_maps(inputs))
    new_states = np.empty((N_MEM, B, DK, DV), np.float32)
    readouts = np.empty((N_MEM, B, DV), np.float32)
    for c, out in enumerate(res.results):
        sl = slice(c * BL, (c + 1) * BL)
        new_states[:, sl] = out["new_states"].reshape(N_MEM, BL, DK, DV)
        readouts[:, sl] = out["readouts"].reshape(N_MEM, BL, DV)
    return new_states, readouts


# revision 23
# speedup vs baseline: 1645.2100x; 1645.2100x over previous
"""Trainium2 Bass kernel for one Mixture-of-Memory (MOM) step.

Reference computation (per memory slot n, batch b):
    outer      = key[b] (outer) value[b]                    # [Dk, Dv]
    new_states[n,b] = lam[b,n] * states[n,b] + (rho[b,n]*alpha[b]) * outer
    readouts[n,b]   = query[b]^T @ new_states[n,b]          # [Dv]

Sharding: batch dim B=64 split across 8 NeuronCores (8 batches/core).
Every op is batch-pointwise, so there is no cross-core communication.

Per-core layout (pair = n*8 + b_local, n-major so it matches the DRAM
order of states/readouts):
  - states arrive as [64, 256, 256]; each (n,b) matrix is loaded as an
    SBUF tile slice [128 partitions, 2*256] with partition = k%128 and
    free = (k_half, v).  One 2 MB DMA per n covers all 8 local batches.
  - The k (outer) x v products for the 8 local batches are computed once
    on the PE (contraction dim 1) and kept in SBUF.
  - Update: ACT does lam*M_old (activation Copy with per-partition
    scale), DVE adds w*outer on top (scalar_tensor_tensor).
  - Readout: 2 accumulating matmuls per pair with lhsT = q^T column
    (PSUM [1,256]), copied to an SBUF staging row on ACT.
"""

import numpy as np

N_MEM = 8
B = 64
DK = 256
DV = 256
N_CORES = 8
BL = B // N_CORES  # local batch per core = 8
PAIRS = N_MEM * BL  # 64
P = 128  # SBUF partitions
KH = DK // P  # 2 k-halves
SMALLS = 2 * BL * DK + 2 * PAIRS + BL  # packed small-input row length

_CACHE = {}


def _build_nc(repeat=1):
    import concourse.masks as masks
    import concourse.mybir as mybir
    from concourse import bacc
    from concourse.tile import TileContext

    f32 = mybir.dt.float32
    nc = bacc.Bacc("TRN2", target_bir_lowering=False, debug=False,
                   num_devices=N_CORES)

    states_t = nc.dram_tensor("states", [PAIRS, DK, DV], f32, kind="ExternalInput")
    # smalls = [key (8*256) | value (8*256) | rho (64) | lam (64) | alpha (8)]
    smalls_t = nc.dram_tensor("smalls", [1, SMALLS], f32, kind="ExternalInput")
    query_t = nc.dram_tensor("query", [BL, DK], f32, kind="ExternalInput")

    ns_t = nc.dram_tensor("new_states", [PAIRS, DK, DV], f32, kind="ExternalOutput")
    ro_t = nc.dram_tensor("readouts", [PAIRS, DV], f32, kind="ExternalOutput")

    mult = mybir.AluOpType.mult
    add = mybir.AluOpType.add
    act_copy = mybir.ActivationFunctionType.Copy

    HB = BL // 2  # batches per half-iteration

    with TileContext(nc) as tc:
        with (
            tc.tile_pool(name="const", bufs=1) as cpool,
            tc.tile_pool(name="tin", bufs=5) as in_pool,
            tc.tile_pool(name="tout", bufs=5) as out_pool,
            tc.tile_pool(name="rrow", bufs=2) as r_pool,
            tc.tile_pool(name="ps_bc", bufs=1, space="PSUM") as ps_bc_pool,
            tc.tile_pool(name="ps_outer", bufs=2, space="PSUM") as ps_outer_pool,
            tc.tile_pool(name="ps_r", bufs=4, space="PSUM") as ps_r_pool,
        ):
            # ---- small-input prep (ACT HWDGE ring, so the big states
            # DMAs own the SP ring from t=0) ---------------------------
            sm = cpool.tile([1, SMALLS], f32)
            nc.scalar.dma_start(out=sm[0:1, :], in_=smalls_t[:, :])
            key_row = sm[0:1, 0:BL * DK]
            value_row = sm[0:1, BL * DK:2 * BL * DK]
            rho_row = sm[0:1, 2 * BL * DK:2 * BL * DK + PAIRS]
            lam_row = sm[0:1, 2 * BL * DK + PAIRS:2 * BL * DK + 2 * PAIRS]
            alpha_row = sm[0:1, 2 * BL * DK + 2 * PAIRS:2 * BL * DK + 2 * PAIRS + BL]
            # q^T: [128, kh*8+b] = query[b, kh*128+p], via PE transpose
            q_sb = cpool.tile([BL, DK], f32)
            nc.scalar.dma_start(out=q_sb[:, :], in_=query_t[:, :])
            id8 = cpool.tile([BL, BL], f32)
            masks.make_identity(nc, id8[:, :])
            qT = cpool.tile([P, KH * BL], f32)
            ps_q = ps_bc_pool.tile([P, KH * BL], f32, tag="ps_q")
            q_par = q_sb[:, :].rearrange("b (p j) -> b j p", j=KH)
            for j in range(KH):
                nc.tensor.transpose(ps_q[:, j * BL:(j + 1) * BL],
                                    q_par[0:BL, j, :],
                                    id8[0:BL, 0:BL])
            nc.vector.tensor_copy(out=qT[:, :], in_=ps_q[:, :])

            ones_row = cpool.tile([1, P], f32)
            nc.vector.memset(ones_row[0:1, :], 1.0)

            # w[b,n] = rho[b,n] * alpha[b]  (all on partition 0)
            w_row = cpool.tile([1, PAIRS], f32)
            for b in range(BL):
                nc.vector.tensor_scalar_mul(
                    w_row[0:1, b * N_MEM:(b + 1) * N_MEM],
                    rho_row[0:1, b * N_MEM:(b + 1) * N_MEM],
                    alpha_row[0:1, b:b + 1],
                )

            # broadcast w and lam to all 128 partitions via ones-matmul
            ps_bc = ps_bc_pool.tile([P, 2 * PAIRS], f32)
            nc.tensor.matmul(ps_bc[:, 0:PAIRS], lhsT=ones_row[0:1, :],
                             rhs=w_row[0:1, :], start=True, stop=True)
            nc.tensor.matmul(ps_bc[:, PAIRS:2 * PAIRS], lhsT=ones_row[0:1, :],
                             rhs=lam_row[0:1, :], start=True, stop=True)
            bc_sb = cpool.tile([P, 2 * PAIRS], f32)
            nc.vector.tensor_copy(out=bc_sb[:, :], in_=ps_bc[:, :])

            # unscaled outer products per local batch: [128, (b, kh*v)]
            outer_sb = cpool.tile([P, BL * KH * DV], f32)
            key_par = key_row.rearrange("o (b p j) -> o b j p", b=BL, j=KH)
            for b in range(BL):
                ps_o = ps_outer_pool.tile([P, KH * DV], f32)
                for j in range(KH):
                    nc.tensor.matmul(
                        ps_o[:, j * DV:(j + 1) * DV],
                        lhsT=key_par[0:1, b, j, :],
                        rhs=value_row[0:1, b * DV:(b + 1) * DV],
                        start=True, stop=True)
                nc.vector.tensor_copy(out=outer_sb[:, b * KH * DV:(b + 1) * KH * DV],
                                      in_=ps_o[:, :])

            # ---- main loop over (memory slot, batch-half) -------------
            for rep in range(repeat):
              for n in range(N_MEM):
                rrow = r_pool.tile([1, BL * DV], f32)
                for h in range(2):
                    b0 = h * HB
                    last = (rep == repeat - 1 and n == N_MEM - 1 and h == 1)
                    tin = in_pool.tile([P, HB * KH * DV], f32)
                    nc.sync.dma_start(
                        out=tin[:, :].rearrange("p (b x) -> p b x", b=HB),
                        in_=states_t[n * BL + b0:n * BL + b0 + HB]
                            .rearrange("b (p j) v -> p b (j v)", p=P, j=KH))
                    if not last:
                        tout = out_pool.tile([P, HB * KH * DV], f32)
                    for bi in range(HB):
                        b = b0 + bi
                        col = b * N_MEM + n
                        if last:
                            # per-b tiles so each 256 KB store can start as
                            # soon as its slice is computed (faster drain)
                            tout = out_pool.tile([P, KH * DV], f32,
                                                 tag="tout_last")
                            sl = slice(0, KH * DV)
                        else:
                            sl = slice(bi * KH * DV, (bi + 1) * KH * DV)
                        osl = slice(b * KH * DV, (b + 1) * KH * DV)
                        # tout = lam * M_old  (ScalarE)
                        nc.scalar.activation(
                            tout[:, sl], tin[:, bi * KH * DV:(bi + 1) * KH * DV],
                            act_copy,
                            scale=bc_sb[:, PAIRS + col:PAIRS + col + 1])
                        # tout += w * outer  (VectorE)
                        nc.vector.scalar_tensor_tensor(
                            out=tout[:, sl], in0=outer_sb[:, osl],
                            scalar=bc_sb[:, col:col + 1], in1=tout[:, sl],
                            op0=mult, op1=add)
                        # readout: q^T @ M_new  -> psum [1, 256]
                        ps_r = ps_r_pool.tile([1, DV], f32)
                        for kh in range(KH):
                            nc.tensor.matmul(
                                ps_r[0:1, :],
                                lhsT=qT[:, kh * BL + b: kh * BL + b + 1],
                                rhs=tout[:, sl.start + kh * DV:
                                         sl.start + (kh + 1) * DV],
                                start=(kh == 0), stop=(kh == KH - 1))
                        nc.scalar.copy(out=rrow[0:1, b * DV:(b + 1) * DV],
                                       in_=ps_r[0:1, :])
                        if last:
                            nc.sync.dma_start(
                                out=ns_t[n * BL + b]
                                    .rearrange("(p j) v -> p (j v)", p=P),
                                in_=tout[:, :])
                    if not last:
                        nc.sync.dma_start(
                            out=ns_t[n * BL + b0:n * BL + b0 + HB]
                                .rearrange("b (p j) v -> p b (j v)", p=P, j=KH),
                            in_=tout[:, :].rearrange("p (b x) -> p b x", b=HB))
                nc.scalar.dma_start(
                    out=ro_t[n * BL:(n + 1) * BL].rearrange("b v -> (b v)")[None, :],
                    in_=rrow[0:1, :])
    nc.compile()
    return nc


def _get_nc():
    if "nc" not in _CACHE:
        _CACHE["nc"] = _build_nc()
    return _CACHE["nc"]


def _in_maps(inputs):
    states = np.ascontiguousarray(np.asarray(inputs["states"], dtype=np.float32))
    key = np.asarray(inputs["key"], dtype=np.float32)
    value = np.asarray(inputs["value"], dtype=np.float32)
    alpha = np.asarray(inputs["alpha"], dtype=np.float32)
    rho = np.asarray(inputs["rho"], dtype=np.float32)
    lam = np.asarray(inputs["lam"], dtype=np.float32)
    query = np.asarray(inputs["query"], dtype=np.float32)

    maps = []
    for c in range(N_CORES):
        sl = slice(c * BL, (c + 1) * BL)
        smalls = np.concatenate([
            key[sl].ravel(), value[sl].ravel(), rho[sl].ravel(),
            lam[sl].ravel(), alpha[sl].ravel(),
        ]).astype(np.float32).reshape(1, SMALLS)
        maps.append({
            "states": np.ascontiguousarray(states[:, sl]).reshape(PAIRS, DK, DV),
            "smalls": smalls,
            "query": np.ascontiguousarray(query[sl]),
        })
    return maps


def _run(in_maps, **kwargs):
    from concourse.bass_utils import run_bass_kernel_spmd
    nc = _get_nc()
    return run_bass_kernel_spmd(nc, in_maps, core_ids=list(range(N_CORES)),
                                **kwargs)


def kernel(**inputs):
    res = _run(_in_maps(inputs))
    new_states = np.empty((N_MEM, B, DK, DV), np.float32)
    readouts = np.empty((N_MEM, B, DV), np.float32)
    for c, out in enumerate(res.results):
        sl = slice(c * BL, (c + 1) * BL)
        new_states[:, sl] = out["new_states"].reshape(N_MEM, BL, DK, DV)
        readouts[:, sl] = out["readouts"].reshape(N_MEM, BL, DV)
    return new_states, readouts
